# revision 1
# baseline (speedup 1.0000x reference)
"""Conv1d (B=32, C_in=256, L=4096, C_out=512, K=9, stride=1, pad=4) on 8 trn2 cores.

Data-parallel over batch: 4 batches per core; weights/bias broadcast.
Per core: out[b, t, co] = sum_{ci,k} x_pad[b, ci, t+k] * w[co, ci, k] + bias[co]
computed as 18 PSUM-accumulated matmuls per 128-position output tile:
  stationary lhsT = x_pad[ci(128), t(128)]  (slid by k)
  moving    rhs  = w_k[ci(128), co(512)]    (host-pre-transposed to [K, C_in, C_out])
PSUM tile [t(128), co(512)] -> +bias on DVE -> DMA to (B, T, C_out) output.
"""

import numpy as np

B, C_IN, L = 32, 256, 4096
C_OUT, KW = 512, 9
PAD = 4
N_CORES = 8
B_LOC = B // N_CORES  # 4
P = 128
CI_CHUNKS = C_IN // P  # 2
T_TILE = 128
LP = L + 2 * PAD  # 4104
N_TT = L // T_TILE  # 32

# matmul input dtype mode: "f32r" (full-rate), "f32" (exact, 4x slower)
MM_MODE = "f32r"

_cache = {}


def _build_program(repeat=1):
    from contextlib import ExitStack

    import concourse.tile as tile
    from concourse import bacc, mybir

    f32 = mybir.dt.float32
    mm_dt = mybir.dt.float32r if MM_MODE == "f32r" else mybir.dt.float32

    nc = bacc.Bacc("TRN2", debug=False)
    x_d = nc.dram_tensor("x", [B_LOC, C_IN, LP], mm_dt, kind="ExternalInput").ap()
    w_d = nc.dram_tensor("w", [KW, C_IN, C_OUT], mm_dt, kind="ExternalInput").ap()
    b_d = nc.dram_tensor("bias", [C_OUT], f32, kind="ExternalInput").ap()
    o_d = nc.dram_tensor("out", [B_LOC, L, C_OUT], f32, kind="ExternalOutput").ap()

    with tile.TileContext(nc) as tc:
        with ExitStack() as ctx:
            persist = ctx.enter_context(tc.tile_pool(name="persist", bufs=1))
            wt = persist.tile(
                [P, KW * CI_CHUNKS * C_OUT], mm_dt, name="wt", tag="wt"
            )
            bias_sb = persist.tile([P, C_OUT], f32, name="bias_sb", tag="bias")
            xps = [
                persist.tile([P, CI_CHUNKS * LP], mm_dt, name=f"xp{i}", tag=f"xp{i}")
                for i in range(2)
            ]

            psum_pool = ctx.enter_context(
                tc.tile_pool(name="psum", bufs=8, space="PSUM")
            )
            out_pool = ctx.enter_context(tc.tile_pool(name="outs", bufs=6))

            NS = 8  # x DMA slices per (batch, ci-chunk): finer deps, earlier start
            SW = LP // NS  # 513
            assert SW * NS == LP

            def emit_w(k):
                # wt column block (k*2+c) holds w[k, c*128:(c+1)*128, :].
                for c in range(CI_CHUNKS):
                    j = (k * CI_CHUNKS + c) * C_OUT
                    nc.sync.dma_start(
                        out=wt[:, j : j + C_OUT], in_=w_d[k, c * P : (c + 1) * P, :]
                    )

            def emit_x(b, slices=range(NS)):
                xp = xps[b % 2]
                for s in slices:
                    for c in range(CI_CHUNKS):
                        nc.sync.dma_start(
                            out=xp[:, c * LP + s * SW : c * LP + (s + 1) * SW],
                            in_=x_d[b, c * P : (c + 1) * P, s * SW : (s + 1) * SW],
                        )

            # Warm-up matmuls on scratch data: PE ramps to full clock (HAM /
            # p-state) during the initial weight/x DMA wait instead of running
            # the first real groups cold. f32 dtype (memset can't produce
            # fp32r); results land in a rotating psum bank, never read.
            NWARM = 12
            if NWARM:
                warm_sb = persist.tile([P, C_OUT], f32, name="warm_sb", tag="warm")
                nc.gpsimd.memset(warm_sb[:], 1.0)
                warm_ps = psum_pool.tile([P, C_OUT], f32, name="ps")
                for i in range(NWARM):
                    nc.tensor.matmul(
                        warm_ps[:, :P],
                        lhsT=warm_sb[:, :P],
                        rhs=warm_sb[:, :P],
                        start=(i == 0),
                        stop=(i == NWARM - 1),
                    )

            # Emission order shapes DMA priority: first-needed data first —
            # k=0 weights, x slice 0, remaining weights, remaining x slices.
            emit_w(0)
            emit_x(0, slices=[0])
            for k in range(1, KW):
                emit_w(k)
            nc.sync.dma_start(
                out=bias_sb[:], in_=b_d.unsqueeze(0).to_broadcast((P, C_OUT))
            )
            emit_x(0, slices=range(1, NS))

            def body(first=False):
                for b in range(B_LOC):
                    if not (first and b == 0):
                        emit_x(b)
                    xp = xps[b % 2]
                    for ti in range(N_TT):
                        t0 = ti * T_TILE
                        ps = psum_pool.tile([P, C_OUT], f32, name="ps")
                        n_mm = KW * CI_CHUNKS
                        i = 0
                        for k in range(KW):
                            for c in range(CI_CHUNKS):
                                j = (k * CI_CHUNKS + c) * C_OUT
                                nc.tensor.matmul(
                                    ps[:],
                                    lhsT=xp[
                                        :, c * LP + t0 + k : c * LP + t0 + k + T_TILE
                                    ],
                                    rhs=wt[:, j : j + C_OUT],
                                    start=(i == 0),
                                    stop=(i == n_mm - 1),
                                )
                                i += 1
                        ob = out_pool.tile([P, C_OUT], f32, name="ob")
                        nc.vector.tensor_add(ob[:], ps[:], bias_sb[:])
                        nc.sync.dma_start(
                            out=o_d[b, t0 : t0 + T_TILE, :], in_=ob[:]
                        )

            for r in range(repeat):
                body(first=(r == 0))

    nc.compile()
    return nc


def _get_program(repeat=1):
    key = ("nc", repeat)
    if key not in _cache:
        _cache[key] = _build_program(repeat)
    return _cache[key]


def _make_in_maps(x, w, bias):
    wt = np.ascontiguousarray(np.transpose(w, (2, 1, 0)))  # (K, C_in, C_out)
    xp = np.pad(x, ((0, 0), (0, 0), (PAD, PAD)))  # (B, C_in, L+2*PAD)
    return [
        {
            "x": np.ascontiguousarray(xp[c * B_LOC : (c + 1) * B_LOC]),
            "w": wt,
            "bias": bias,
        }
        for c in range(N_CORES)
    ]


def _get_runner():
    """Cached SPMD runner: same bass2jax/PJRT execution path that
    run_bass_kernel_spmd uses under axon, but the jitted executable and the
    (constant) zero output operands are built once and reused per call."""
    if "runner" in _cache:
        return _cache["runner"]

    import jax
    from jax.sharding import Mesh, NamedSharding, PartitionSpec
    from jax.experimental.shard_map import shard_map
    from concourse import mybir
    from concourse.bass2jax import (
        _bass_exec_p,
        install_neuronx_cc_hook,
        partition_id_tensor,
    )

    install_neuronx_cc_hook()
    nc = _get_program()
    partition_name = nc.partition_id_tensor.name if nc.partition_id_tensor else None
    in_names, out_names, out_avals, zero_outs = [], [], [], []
    for alloc in nc.m.functions[0].allocations:
        if not isinstance(alloc, mybir.MemoryLocationSet):
            continue
        name = alloc.memorylocations[0].name
        if alloc.kind == "ExternalInput":
            if name != partition_name:
                in_names.append(name)
        elif alloc.kind == "ExternalOutput":
            shape = tuple(alloc.tensor_shape)
            dtype = mybir.dt.np(alloc.dtype)
            out_names.append(name)
            out_avals.append(jax.core.ShapedArray(shape, dtype))
            zero_outs.append(np.zeros(shape, dtype))
    n_params = len(in_names)
    all_names = in_names + out_names
    if partition_name is not None:
        all_names = all_names + [partition_name]

    def _body(*args):
        extra = [partition_id_tensor()] if partition_name is not None else []
        return tuple(
            _bass_exec_p.bind(
                *(list(args) + extra),
                out_avals=tuple(out_avals),
                in_names=tuple(all_names),
                out_names=tuple(out_names),
                lowering_input_output_aliases=(),
                sim_require_finite=True,
                sim_require_nnan=True,
                nc=nc,
            )
        )

    devices = jax.devices()[:N_CORES]
    mesh = Mesh(np.asarray(devices), ("core",))
    sharding = NamedSharding(mesh, PartitionSpec("core"))
    fn = jax.jit(
        shard_map(
            _body,
            mesh=mesh,
            in_specs=(PartitionSpec("core"),) * (n_params + len(out_names)),
            out_specs=(PartitionSpec("core"),) * len(out_names),
            check_rep=False,
        )
    )
    # Zero "output" operands: required custom-call inputs; the kernel writes
    # every output element, so these can be device-resident constants.
    zeros_dev = [
        jax.device_put(np.concatenate([z] * N_CORES, axis=0), sharding)
        for z in zero_outs
    ]
    _cache["runner"] = (fn, in_names, out_names, zeros_dev, sharding)
    return _cache["runner"]


def kernel(**inputs):
    x = np.asarray(inputs["x"], dtype=np.float32)
    w = np.asarray(inputs["weight"], dtype=np.float32)
    bias = np.asarray(inputs["bias"], dtype=np.float32)

    try:
        import jax

        fn, in_names, out_names, zeros_dev, sharding = _get_runner()
        # Global (concat-across-cores) operands; shard c along axis 0 is core
        # c's slice: x -> batches 4c..4c+3 (padded), w/bias -> replicated.
        wt = np.ascontiguousarray(np.transpose(w, (2, 1, 0)))  # (K, C_in, C_out)
        glob = {
            "x": np.pad(x, ((0, 0), (0, 0), (PAD, PAD))),
            "w": np.concatenate([wt] * N_CORES, axis=0),
            "bias": np.concatenate([bias] * N_CORES, axis=0),
        }
        dev_in = [jax.device_put(glob[nm], sharding) for nm in in_names]
        r = fn(*dev_in, *zeros_dev)
        out = np.asarray(r[out_names.index("out")])
        return out.reshape(B, L, C_OUT)
    except Exception:
        # Fallback: the stock SPMD runner (same program, per-core in_maps).
        from concourse.bass_utils import run_bass_kernel_spmd

        nc = _get_program()
        res = run_bass_kernel_spmd(
            nc, _make_in_maps(x, w, bias), list(range(N_CORES))
        )
        return np.concatenate(
            [res.results[c]["out"] for c in range(N_CORES)], axis=0
        )



# revision 10
# speedup vs baseline: 2.0047x; 2.0047x over previous
"""Conv1d (B=32, C_in=256, L=4096, C_out=512, K=9, stride=1, pad=4) on 8 trn2 cores.

Data-parallel over batch: 4 batches per core. Winograd-style factorization:
the K=9 kernel splits into 3 sub-filters of 3 taps (k = 3j+k'); each sub-conv
uses F(3,3) Toom-Cook (5 points, tile stride 3), and because the sub-filter
shift (3) equals the tile stride, all 3 sub-filters share one transform domain
where the j-shift becomes a 3-tap conv over tile index. MACs drop to 5/9 of
direct conv.

Per core pipeline (all matmuls bf16 operands, fp32 PSUM):
  input transform (PE):  stationary x^T window [t(128), ci(128)], moving
                         S [t(128), (p,i_loc)(210)] -> V-block [ci, p*42+i]
  V drain (DVE/Pool/Act): PSUM -> SBUF bf16, per-p globally contiguous over i
  stage 2 (PE):          stationary V_p[ci, i0+j : i0+j+128] slid by j=0..2,
                         moving U[p,j][ci, co(512)] -> Y_p [i(128), co(512)]
                         (30 PSUM-accumulated matmuls per 128-tile block)
  A-stage (DVE+Pool):    out_u[i,co] = sum_p AT[u,p] Y_p + bias via fused
                         scalar_tensor_tensor folds, Y_p freed right away
  out DMA:               [i, (u,co)] tile -> DRAM out[b, 3i+u, co] (6KB rows)

F(3,3) matrices (points 0,1,-1,2 + inf; BT rows scaled integer, the 1/2 1/6
factors live in the fp32 A-stage):
  BT = [[2,-1,-2, 1,0],[0,2,1,-1,0],[0,-2,3,-1,0],[0,-1,0,1,0],[0,2,-1,-2,1]]
  G  = [[1,0,0],[1,1,1],[1,-1,1],[1,2,4],[0,0,1]]
  AT = [[1/2,1/2,1/6,1/6,0],[0,1/2,-1/6,1/3,0],[0,1/2,1/6,2/3,1]]
"""

import numpy as np

B, C_IN, L = 32, 256, 4096
C_OUT, KW = 512, 9
PAD = 4
N_CORES = 8
B_LOC = B // N_CORES  # 4
P = 128
CI_CHUNKS = C_IN // P  # 2

NP5 = 5          # winograd points
NJ = 3           # sub-filter taps over tile index
M3 = 3           # outputs per tile
WTILES = 42      # V tiles computed per transform window (i_loc 0..41)
WSTRIDE = 40     # window stride in tiles (2-tile overlap for j-shift reuse)
NWIN = 36        # transform windows per batch row
SCOLS = NP5 * WTILES  # 210 moving columns in transform matmul
NT_V = WSTRIDE * (NWIN - 1) + WTILES  # 1442 V tiles computed (incl overlap)
NVC = 1444       # V SBUF columns per (c,p) (pad a bit)
TX = 4352        # padded x^T rows: last window reads t in [4200, 4328)
NBLK = 11        # stage-2 blocks of 128 tiles per batch row
NI_OUT = 1366    # out tiles: i <= 1365 (t = 3i+u <= 4095; i=1365 u=0 only)

BT_W = np.array([[2,-1,-2,1,0],[0,2,1,-1,0],[0,-2,3,-1,0],[0,-1,0,1,0],
                 [0,2,-1,-2,1]], np.float64)
G_W = np.array([[1,0,0],[1,1,1],[1,-1,1],[1,2,4],[0,0,1]], np.float64)
AT_W = np.array([[1/2,1/2,1/6,1/6,0],[0,1/2,-1/6,1/3,0],[0,1/2,1/6,2/3,1]],
                np.float64)

_cache = {}


def _build_program(repeat=1):
    from contextlib import ExitStack

    import concourse.tile as tile
    from concourse import bacc, mybir

    f32 = mybir.dt.float32
    bf16 = mybir.dt.bfloat16
    MUL = mybir.AluOpType.mult
    ADD = mybir.AluOpType.add

    nc = bacc.Bacc("TRN2", debug=False)
    xt_d = nc.dram_tensor("xt", [B_LOC, TX, C_IN], bf16, kind="ExternalInput").ap()
    u_d = nc.dram_tensor(
        "u", [NP5, NJ, CI_CHUNKS, P, C_OUT], bf16, kind="ExternalInput"
    ).ap()
    s_d = nc.dram_tensor("s", [P, SCOLS], bf16, kind="ExternalInput").ap()
    b_d = nc.dram_tensor("bias", [C_OUT], f32, kind="ExternalInput").ap()
    o_d = nc.dram_tensor("out", [B_LOC, L, C_OUT], f32, kind="ExternalOutput").ap()

    with tile.TileContext(nc) as tc:
        with ExitStack() as ctx:
            persist = ctx.enter_context(tc.tile_pool(name="persist", bufs=1))
            s_sb = persist.tile([P, SCOLS], bf16, name="s_sb", tag="s")
            bias_sb = persist.tile([P, C_OUT], f32, name="bias_sb", tag="bias")
            # U tiles: [p][j][c] -> [128, 512]
            u_sb = [
                [
                    [
                        persist.tile(
                            [P, C_OUT], bf16, name=f"u{p}{j}{c}", tag=f"u{p}{j}{c}"
                        )
                        for c in range(CI_CHUNKS)
                    ]
                    for j in range(NJ)
                ]
                for p in range(NP5)
            ]
            # V tensors: [parity][c][p] -> [128, NVC] bf16, i globally contiguous
            v_sb = [
                [
                    [
                        persist.tile(
                            [P, NVC], bf16, name=f"v{par}{c}{p}", tag=f"v{par}{c}{p}"
                        )
                        for p in range(NP5)
                    ]
                    for c in range(CI_CHUNKS)
                ]
                for par in range(2)
            ]

            xw_pool = ctx.enter_context(tc.tile_pool(name="xw", bufs=4))
            psum_t = ctx.enter_context(
                tc.tile_pool(name="psum_t", bufs=5, space="PSUM")
            )
            psum_y = ctx.enter_context(
                tc.tile_pool(name="psum_y", bufs=3, space="PSUM")
            )
            e_pool = ctx.enter_context(tc.tile_pool(name="e", bufs=6))
            out_pool = ctx.enter_context(tc.tile_pool(name="outs", bufs=2))

            # Warm-up matmuls: ramp PE p-state while initial DMAs land.
            NWARM = 12
            warm_sb = persist.tile([P, C_OUT], f32, name="warm_sb", tag="warm")
            nc.gpsimd.memset(warm_sb[:], 1.0)
            warm_ps = psum_y.tile([P, C_OUT], f32, name="ps")
            for i in range(NWARM):
                nc.tensor.matmul(
                    warm_ps[:, :P],
                    lhsT=warm_sb[:, :P],
                    rhs=warm_sb[:, :P],
                    start=(i == 0),
                    stop=(i == NWARM - 1),
                )

            nc.sync.dma_start(out=s_sb[:], in_=s_d[:, :])
            for p in range(NP5):
                for j in range(NJ):
                    for c in range(CI_CHUNKS):
                        nc.sync.dma_start(
                            out=u_sb[p][j][c][:], in_=u_d[p, j, c, :, :]
                        )
            nc.sync.dma_start(
                out=bias_sb[:], in_=b_d.unsqueeze(0).to_broadcast((P, C_OUT))
            )

            def _drain(p, out_ap, in_ap):
                # gpsimd can't touch PSUM on HW; split drains vector/scalar.
                # p-keyed split keeps same-p drains ordered on one engine.
                if p < 2:
                    nc.vector.tensor_copy(out=out_ap, in_=in_ap)
                else:
                    nc.scalar.copy(out=out_ap, in_=in_ap)

            def emit_transform_pair(b, wp):
                """Transform windows 2wp, 2wp+1 of batch b: 2 matmuls per
                ci-chunk into one PSUM tile, then 10 per-p drains."""
                par = b % 2
                xws = []
                for wi in range(2):
                    w = 2 * wp + wi
                    t0 = WSTRIDE * M3 * w  # 120*w
                    xw = xw_pool.tile([P, C_IN], bf16, name="xw")
                    nc.sync.dma_start(out=xw[:], in_=xt_d[b, t0 : t0 + P, :])
                    xws.append(xw)
                for c in range(CI_CHUNKS):
                    pt = psum_t.tile([P, 2 * SCOLS], f32, name="pt")
                    for wi in range(2):
                        nc.tensor.matmul(
                            pt[:, wi * SCOLS : (wi + 1) * SCOLS],
                            lhsT=xws[wi][:, c * P : (c + 1) * P],
                            rhs=s_sb[:],
                            start=True,
                            stop=True,
                        )
                    for wi in range(2):
                        w = 2 * wp + wi
                        # skip the 2-tile overlap with the previous window
                        skip = 0 if w == 0 else 2
                        for p in range(NP5):
                            _drain(
                                p,
                                v_sb[par][c][p][
                                    :,
                                    WSTRIDE * w + skip : WSTRIDE * w + WTILES,
                                ],
                                pt[
                                    :,
                                    wi * SCOLS + p * WTILES + skip : wi * SCOLS
                                    + (p + 1) * WTILES,
                                ],
                            )

            def emit_block(b, g):
                """Stage-2 + A-stage + out DMA for block g (tiles 128g..128g+127)."""
                par = b % 2
                i0 = P * g
                e0 = e_pool.tile([P, C_OUT], f32, name="e0")
                e1 = e_pool.tile([P, C_OUT], f32, name="e1")
                e2 = e_pool.tile([P, C_OUT], f32, name="e2")
                ot = out_pool.tile([P, M3 * C_OUT], f32, name="ot")
                ot3 = ot.rearrange("p (u c) -> p u c", u=M3)
                for p in range(NP5):
                    ps = psum_y.tile([P, C_OUT], f32, name="ps")
                    mm = 0
                    for j in range(NJ):
                        for c in range(CI_CHUNKS):
                            nc.tensor.matmul(
                                ps[:],
                                lhsT=v_sb[par][c][p][:, i0 + j : i0 + j + P],
                                rhs=u_sb[p][j][c][:],
                                start=(mm == 0),
                                stop=(mm == NJ * CI_CHUNKS - 1),
                            )
                            mm += 1
                    # fold Y_p immediately (frees the PSUM bank). Only the
                    # vector engine may read PSUM among DVE-likes; gpsimd
                    # handles SBUF-only folds via Y3/Y4 staged through SBUF.
                    if p == 0:
                        nc.vector.scalar_tensor_tensor(
                            out=e0[:], in0=ps[:], scalar=0.5, in1=bias_sb[:],
                            op0=MUL, op1=ADD,
                        )
                    elif p == 1:
                        nc.vector.scalar_tensor_tensor(
                            out=e0[:], in0=ps[:], scalar=0.5, in1=e0[:],
                            op0=MUL, op1=ADD,
                        )
                        nc.vector.scalar_tensor_tensor(
                            out=e1[:], in0=ps[:], scalar=0.5, in1=bias_sb[:],
                            op0=MUL, op1=ADD,
                        )
                        nc.vector.scalar_tensor_tensor(
                            out=e2[:], in0=ps[:], scalar=0.5, in1=bias_sb[:],
                            op0=MUL, op1=ADD,
                        )
                    elif p == 2:
                        nc.vector.scalar_tensor_tensor(
                            out=e0[:], in0=ps[:], scalar=1.0 / 6, in1=e0[:],
                            op0=MUL, op1=ADD,
                        )
                        nc.vector.scalar_tensor_tensor(
                            out=e1[:], in0=ps[:], scalar=-1.0 / 6, in1=e1[:],
                            op0=MUL, op1=ADD,
                        )
                        nc.vector.scalar_tensor_tensor(
                            out=e2[:], in0=ps[:], scalar=1.0 / 6, in1=e2[:],
                            op0=MUL, op1=ADD,
                        )
                    elif p == 3:
                        nc.vector.scalar_tensor_tensor(
                            out=ot3[:, 0, :], in0=ps[:], scalar=1.0 / 6, in1=e0[:],
                            op0=MUL, op1=ADD,
                        )
                        nc.vector.scalar_tensor_tensor(
                            out=ot3[:, 1, :], in0=ps[:], scalar=1.0 / 3, in1=e1[:],
                            op0=MUL, op1=ADD,
                        )
                        nc.vector.scalar_tensor_tensor(
                            out=e2[:], in0=ps[:], scalar=2.0 / 3, in1=e2[:],
                            op0=MUL, op1=ADD,
                        )
                    else:
                        nc.vector.scalar_tensor_tensor(
                            out=ot3[:, 2, :], in0=ps[:], scalar=1.0, in1=e2[:],
                            op0=MUL, op1=ADD,
                        )
                # DMA out: valid i rows only
                if g < NBLK - 1:
                    nrow = P
                    o_slice = o_d[b, M3 * i0 : M3 * (i0 + nrow), :]
                    nc.sync.dma_start(
                        out=o_slice.rearrange("(i u) c -> i u c", u=M3),
                        in_=ot3[:nrow, :, :],
                    )
                else:
                    nrow = NI_OUT - 1 - i0  # 85 full rows (i <= 1364)
                    o_slice = o_d[b, M3 * i0 : M3 * (i0 + nrow), :]
                    nc.sync.dma_start(
                        out=o_slice.rearrange("(i u) c -> i u c", u=M3),
                        in_=ot3[:nrow, :, :],
                    )
                    # tail: t = 4095 = 3*1365 + 0
                    nc.sync.dma_start(
                        out=o_d[b, L - 1 : L, :],
                        in_=ot3[nrow : nrow + 1, 0, :],
                    )

            def emit_transform_batch(b, start_wp, n_wp):
                for wp in range(start_wp, min(start_wp + n_wp, NWIN // 2)):
                    emit_transform_pair(b, wp)

            def body(first, last):
                for b in range(B_LOC):
                    if first and b == 0:
                        emit_transform_batch(0, 0, NWIN // 2)
                    nxt = (b + 1) % B_LOC
                    skip_next = last and b == B_LOC - 1
                    for g in range(NBLK):
                        emit_block(b, g)
                        if not skip_next:
                            # 18 transform pairs spread over 11 blocks
                            w0 = (18 * g) // NBLK
                            w1 = (18 * (g + 1)) // NBLK
                            emit_transform_batch(nxt, w0, w1 - w0)

            for r in range(repeat):
                body(first=(r == 0), last=(r == repeat - 1))

    nc.compile()
    return nc


def _get_program(repeat=1):
    key = ("nc", repeat)
    if key not in _cache:
        _cache[key] = _build_program(repeat)
    return _cache[key]


def _host_prep(x, w, bias):
    import ml_dtypes

    bf = ml_dtypes.bfloat16
    # x^T: pad to [B, TX, C_IN] bf16, t-major
    xpad = np.zeros((B, TX, C_IN), np.float32)
    xpad[:, PAD : PAD + L, :] = np.transpose(x, (0, 2, 1))
    xt = xpad.astype(bf)
    # U[p,j,c,ci,co] = sum_k G[p,k] w[co, ci, 3j+k]
    wr = w.reshape(C_OUT, C_IN, NJ, 3).astype(np.float64)
    U = np.einsum("pk,ocjk->pjco", G_W, wr)  # (5, 3, C_IN, C_OUT)
    U = U.reshape(NP5, NJ, CI_CHUNKS, P, C_OUT).astype(bf)
    # S[t_local, p*42+i_loc] = BT[p, t_local - 3*i_loc]
    S = np.zeros((P, SCOLS), np.float64)
    for pp in range(NP5):
        for il in range(WTILES):
            for q in range(NP5):
                t_local = 3 * il + q
                if t_local < P:
                    S[t_local, pp * WTILES + il] = BT_W[pp, q]
    S = S.astype(bf)
    return xt, U, S, bias.astype(np.float32)


def _make_in_maps(x, w, bias):
    xt, U, S, bias_f = _host_prep(
        np.asarray(x, np.float32), np.asarray(w, np.float32),
        np.asarray(bias, np.float32),
    )
    return [
        {
            "xt": np.ascontiguousarray(xt[c * B_LOC : (c + 1) * B_LOC]),
            "u": U,
            "s": S,
            "bias": bias_f,
        }
        for c in range(N_CORES)
    ]


def _get_runner():
    """Cached SPMD runner: same bass2jax/PJRT execution path that
    run_bass_kernel_spmd uses under axon, but the jitted executable and the
    (constant) zero output operands are built once and reused per call."""
    if "runner" in _cache:
        return _cache["runner"]

    import jax
    from jax.sharding import Mesh, NamedSharding, PartitionSpec
    from jax.experimental.shard_map import shard_map
    from concourse import mybir
    from concourse.bass2jax import (
        _bass_exec_p,
        install_neuronx_cc_hook,
        partition_id_tensor,
    )

    install_neuronx_cc_hook()
    nc = _get_program()
    partition_name = nc.partition_id_tensor.name if nc.partition_id_tensor else None
    in_names, out_names, out_avals, zero_outs = [], [], [], []
    for alloc in nc.m.functions[0].allocations:
        if not isinstance(alloc, mybir.MemoryLocationSet):
            continue
        name = alloc.memorylocations[0].name
        if alloc.kind == "ExternalInput":
            if name != partition_name:
                in_names.append(name)
        elif alloc.kind == "ExternalOutput":
            shape = tuple(alloc.tensor_shape)
            dtype = mybir.dt.np(alloc.dtype)
            out_names.append(name)
            out_avals.append(jax.core.ShapedArray(shape, dtype))
            zero_outs.append(np.zeros(shape, dtype))
    n_params = len(in_names)
    all_names = in_names + out_names
    if partition_name is not None:
        all_names = all_names + [partition_name]

    def _body(*args):
        extra = [partition_id_tensor()] if partition_name is not None else []
        return tuple(
            _bass_exec_p.bind(
                *(list(args) + extra),
                out_avals=tuple(out_avals),
                in_names=tuple(all_names),
                out_names=tuple(out_names),
                lowering_input_output_aliases=(),
                sim_require_finite=True,
                sim_require_nnan=True,
                nc=nc,
            )
        )

    devices = jax.devices()[:N_CORES]
    mesh = Mesh(np.asarray(devices), ("core",))
    sharding = NamedSharding(mesh, PartitionSpec("core"))
    fn = jax.jit(
        shard_map(
            _body,
            mesh=mesh,
            in_specs=(PartitionSpec("core"),) * (n_params + len(out_names)),
            out_specs=(PartitionSpec("core"),) * len(out_names),
            check_rep=False,
        )
    )
    zeros_dev = [
        jax.device_put(np.concatenate([z] * N_CORES, axis=0), sharding)
        for z in zero_outs
    ]
    _cache["runner"] = (fn, in_names, out_names, zeros_dev, sharding)
    return _cache["runner"]


def kernel(**inputs):
    x = np.asarray(inputs["x"], dtype=np.float32)
    w = np.asarray(inputs["weight"], dtype=np.float32)
    bias = np.asarray(inputs["bias"], dtype=np.float32)

    try:
        import jax

        fn, in_names, out_names, zeros_dev, sharding = _get_runner()
        in_maps = _make_in_maps(x, w, bias)
        glob = {
            nm: np.concatenate([np.asarray(in_maps[c][nm]) for c in range(N_CORES)], axis=0)
            for nm in in_names
        }
        dev_in = [jax.device_put(glob[nm], sharding) for nm in in_names]
        r = fn(*dev_in, *zeros_dev)
        out = np.asarray(r[out_names.index("out")])
        return out.reshape(B, L, C_OUT)
    except Exception:
        # Fallback: the stock SPMD runner (same program, per-core in_maps).
        from concourse.bass_utils import run_bass_kernel_spmd

        nc = _get_program()
        res = run_bass_kernel_spmd(
            nc, _make_in_maps(x, w, bias), list(range(N_CORES))
        )
        return np.concatenate(
            [res.results[c]["out"] for c in range(N_CORES)], axis=0
        )


# revision 12
# speedup vs baseline: 2.1904x; 1.0927x over previous
"""Conv1d (B=32, C_in=256, L=4096, C_out=512, K=9, stride=1, pad=4) on 8 trn2 cores.

Data-parallel over batch: 4 batches per core. Winograd-style factorization:
the K=9 kernel splits into 3 sub-filters of 3 taps (k = 3j+k'); each sub-conv
uses F(3,3) Toom-Cook (5 points, tile stride 3), and because the sub-filter
shift (3) equals the tile stride, all 3 sub-filters share one transform domain
where the j-shift becomes a 3-tap conv over tile index. MACs drop to 5/9 of
direct conv.

Per core pipeline (all matmuls bf16 operands, fp32 PSUM):
  input transform (PE):  stationary x^T window [t(128), ci(128)], moving
                         S [t(128), (p,i_loc)(210)] -> V-block [ci, p*42+i]
  V drain (DVE/Pool/Act): PSUM -> SBUF bf16, per-p globally contiguous over i
  stage 2 (PE):          stationary V_p[ci, i0+j : i0+j+128] slid by j=0..2,
                         moving U[p,j][ci, co(512)] -> Y_p [i(128), co(512)]
                         (30 PSUM-accumulated matmuls per 128-tile block)
  A-stage (DVE+Pool):    out_u[i,co] = sum_p AT[u,p] Y_p + bias via fused
                         scalar_tensor_tensor folds, Y_p freed right away
  out DMA:               [i, (u,co)] tile -> DRAM out[b, 3i+u, co] (6KB rows)

F(3,3) matrices (points 0,1,-1,2 + inf; BT rows scaled integer, the 1/2 1/6
factors live in the fp32 A-stage):
  BT = [[2,-1,-2, 1,0],[0,2,1,-1,0],[0,-2,3,-1,0],[0,-1,0,1,0],[0,2,-1,-2,1]]
  G  = [[1,0,0],[1,1,1],[1,-1,1],[1,2,4],[0,0,1]]
  AT = [[1/2,1/2,1/6,1/6,0],[0,1/2,-1/6,1/3,0],[0,1/2,1/6,2/3,1]]
"""

import numpy as np

B, C_IN, L = 32, 256, 4096
C_OUT, KW = 512, 9
PAD = 4
N_CORES = 8
B_LOC = B // N_CORES  # 4
P = 128
CI_CHUNKS = C_IN // P  # 2

NP5 = 5          # winograd points
NJ = 3           # sub-filter taps over tile index
M3 = 3           # outputs per tile
WTILES = 42      # V tiles computed per transform window (i_loc 0..41)
WSTRIDE = 40     # window stride in tiles (2-tile overlap for j-shift reuse)
NWIN = 36        # transform windows per batch row
SCOLS = NP5 * WTILES  # 210 moving columns in transform matmul
NT_V = WSTRIDE * (NWIN - 1) + WTILES  # 1442 V tiles computed (incl overlap)
NVC = 1444       # V SBUF columns per (c,p) (pad a bit)
TX = 4352        # padded x^T rows: last window reads t in [4200, 4328)
NBLK = 11        # stage-2 blocks of 128 tiles per batch row
NI_OUT = 1366    # out tiles: i <= 1365 (t = 3i+u <= 4095; i=1365 u=0 only)

BT_W = np.array([[2,-1,-2,1,0],[0,2,1,-1,0],[0,-2,3,-1,0],[0,-1,0,1,0],
                 [0,2,-1,-2,1]], np.float64)
G_W = np.array([[1,0,0],[1,1,1],[1,-1,1],[1,2,4],[0,0,1]], np.float64)
AT_W = np.array([[1/2,1/2,1/6,1/6,0],[0,1/2,-1/6,1/3,0],[0,1/2,1/6,2/3,1]],
                np.float64)

_cache = {}


def _build_program(repeat=1):
    from contextlib import ExitStack

    import concourse.tile as tile
    from concourse import bacc, mybir

    f32 = mybir.dt.float32
    bf16 = mybir.dt.bfloat16
    MUL = mybir.AluOpType.mult
    ADD = mybir.AluOpType.add

    nc = bacc.Bacc("TRN2", debug=False)
    xt_d = nc.dram_tensor("xt", [B_LOC, TX, C_IN], bf16, kind="ExternalInput").ap()
    u_d = nc.dram_tensor(
        "u", [NP5, NJ, CI_CHUNKS, P, C_OUT], bf16, kind="ExternalInput"
    ).ap()
    s_d = nc.dram_tensor("s", [P, SCOLS], bf16, kind="ExternalInput").ap()
    b_d = nc.dram_tensor("bias", [C_OUT], f32, kind="ExternalInput").ap()
    o_d = nc.dram_tensor("out", [B_LOC, L, C_OUT], f32, kind="ExternalOutput").ap()

    with tile.TileContext(nc) as tc:
        with ExitStack() as ctx:
            persist = ctx.enter_context(tc.tile_pool(name="persist", bufs=1))
            s_sb = persist.tile([P, SCOLS], bf16, name="s_sb", tag="s")
            bias_sb = persist.tile([P, C_OUT], f32, name="bias_sb", tag="bias")
            # U tiles: [p][j][c] -> [128, 512]
            u_sb = [
                [
                    [
                        persist.tile(
                            [P, C_OUT], bf16, name=f"u{p}{j}{c}", tag=f"u{p}{j}{c}"
                        )
                        for c in range(CI_CHUNKS)
                    ]
                    for j in range(NJ)
                ]
                for p in range(NP5)
            ]
            # V tensors: [parity][c][p] -> [128, NVC] bf16, i globally contiguous
            v_sb = [
                [
                    [
                        persist.tile(
                            [P, NVC], bf16, name=f"v{par}{c}{p}", tag=f"v{par}{c}{p}"
                        )
                        for p in range(NP5)
                    ]
                    for c in range(CI_CHUNKS)
                ]
                for par in range(2)
            ]

            xw_pool = ctx.enter_context(tc.tile_pool(name="xw", bufs=4))
            psum_t = ctx.enter_context(
                tc.tile_pool(name="psum_t", bufs=5, space="PSUM")
            )
            psum_y = ctx.enter_context(
                tc.tile_pool(name="psum_y", bufs=3, space="PSUM")
            )
            e_pool = ctx.enter_context(tc.tile_pool(name="e", bufs=6))
            out_pool = ctx.enter_context(tc.tile_pool(name="outs", bufs=2))

            # Warm-up matmuls: ramp PE p-state while initial DMAs land.
            NWARM = 12
            warm_sb = persist.tile([P, C_OUT], f32, name="warm_sb", tag="warm")
            nc.gpsimd.memset(warm_sb[:], 1.0)
            warm_ps = psum_y.tile([P, C_OUT], f32, name="ps")
            for i in range(NWARM):
                nc.tensor.matmul(
                    warm_ps[:, :P],
                    lhsT=warm_sb[:, :P],
                    rhs=warm_sb[:, :P],
                    start=(i == 0),
                    stop=(i == NWARM - 1),
                )

            nc.sync.dma_start(out=s_sb[:], in_=s_d[:, :])
            for p in range(NP5):
                for j in range(NJ):
                    for c in range(CI_CHUNKS):
                        nc.sync.dma_start(
                            out=u_sb[p][j][c][:], in_=u_d[p, j, c, :, :]
                        )
            nc.sync.dma_start(
                out=bias_sb[:], in_=b_d.unsqueeze(0).to_broadcast((P, C_OUT))
            )

            def _drain(p, out_ap, in_ap):
                # gpsimd can't touch PSUM on HW; split drains vector/scalar.
                # p-keyed split keeps same-p drains ordered on one engine.
                if p < 2:
                    nc.vector.tensor_copy(out=out_ap, in_=in_ap)
                else:
                    nc.scalar.copy(out=out_ap, in_=in_ap)

            def emit_transform_pair(b, wp):
                """Transform windows 2wp, 2wp+1 of batch b: 2 matmuls per
                ci-chunk into one PSUM tile, then 10 per-p drains."""
                par = b % 2
                xws = []
                for wi in range(2):
                    w = 2 * wp + wi
                    t0 = WSTRIDE * M3 * w  # 120*w
                    xw = xw_pool.tile([P, C_IN], bf16, name="xw")
                    nc.sync.dma_start(out=xw[:], in_=xt_d[b, t0 : t0 + P, :])
                    xws.append(xw)
                for c in range(CI_CHUNKS):
                    pt = psum_t.tile([P, 2 * SCOLS], f32, name="pt")
                    pt3 = pt.rearrange("p (wi s) -> p wi s", wi=2)
                    for wi in range(2):
                        nc.tensor.matmul(
                            pt[:, wi * SCOLS : (wi + 1) * SCOLS],
                            lhsT=xws[wi][:, c * P : (c + 1) * P],
                            rhs=s_sb[:],
                            start=True,
                            stop=True,
                        )
                    if wp == 0:
                        # first pair: window 0 keeps its leading 2 tiles
                        for wi in range(2):
                            skip = 0 if wi == 0 else 2
                            for p in range(NP5):
                                _drain(
                                    p,
                                    v_sb[par][c][p][
                                        :,
                                        WSTRIDE * wi + skip : WSTRIDE * wi
                                        + WTILES,
                                    ],
                                    pt3[
                                        :,
                                        wi,
                                        p * WTILES + skip : (p + 1) * WTILES,
                                    ],
                                )
                    else:
                        # both windows skip their 2-tile overlap: the two
                        # 40-col segments are contiguous in V -> one drain
                        v0 = WSTRIDE * 2 * wp + 2
                        for p in range(NP5):
                            _drain(
                                p,
                                v_sb[par][c][p][
                                    :, v0 : v0 + 2 * WSTRIDE
                                ].rearrange("q (a s) -> q a s", a=2),
                                pt3[
                                    :, :, p * WTILES + 2 : (p + 1) * WTILES
                                ],
                            )

            def emit_block(b, g):
                """Stage-2 + A-stage + out DMA for block g (tiles 128g..128g+127)."""
                par = b % 2
                i0 = P * g
                e0 = e_pool.tile([P, C_OUT], f32, name="e0")
                e1 = e_pool.tile([P, C_OUT], f32, name="e1")
                e2 = e_pool.tile([P, C_OUT], f32, name="e2")
                ot = out_pool.tile([P, M3 * C_OUT], f32, name="ot")
                ot3 = ot.rearrange("p (u c) -> p u c", u=M3)
                for p in range(NP5):
                    ps = psum_y.tile([P, C_OUT], f32, name="ps")
                    mm = 0
                    for j in range(NJ):
                        for c in range(CI_CHUNKS):
                            nc.tensor.matmul(
                                ps[:],
                                lhsT=v_sb[par][c][p][:, i0 + j : i0 + j + P],
                                rhs=u_sb[p][j][c][:],
                                start=(mm == 0),
                                stop=(mm == NJ * CI_CHUNKS - 1),
                            )
                            mm += 1
                    # fold Y_p immediately (frees the PSUM bank). Only the
                    # vector engine may read PSUM among DVE-likes; gpsimd
                    # handles SBUF-only folds via Y3/Y4 staged through SBUF.
                    if p == 0:
                        nc.vector.scalar_tensor_tensor(
                            out=e0[:], in0=ps[:], scalar=0.5, in1=bias_sb[:],
                            op0=MUL, op1=ADD,
                        )
                    elif p == 1:
                        nc.vector.scalar_tensor_tensor(
                            out=e0[:], in0=ps[:], scalar=0.5, in1=e0[:],
                            op0=MUL, op1=ADD,
                        )
                        nc.vector.scalar_tensor_tensor(
                            out=e1[:], in0=ps[:], scalar=0.5, in1=bias_sb[:],
                            op0=MUL, op1=ADD,
                        )
                        nc.vector.scalar_tensor_tensor(
                            out=e2[:], in0=ps[:], scalar=0.5, in1=bias_sb[:],
                            op0=MUL, op1=ADD,
                        )
                    elif p == 2:
                        nc.vector.scalar_tensor_tensor(
                            out=e0[:], in0=ps[:], scalar=1.0 / 6, in1=e0[:],
                            op0=MUL, op1=ADD,
                        )
                        nc.vector.scalar_tensor_tensor(
                            out=e1[:], in0=ps[:], scalar=-1.0 / 6, in1=e1[:],
                            op0=MUL, op1=ADD,
                        )
                        nc.vector.scalar_tensor_tensor(
                            out=e2[:], in0=ps[:], scalar=1.0 / 6, in1=e2[:],
                            op0=MUL, op1=ADD,
                        )
                    elif p == 3:
                        nc.vector.scalar_tensor_tensor(
                            out=ot3[:, 0, :], in0=ps[:], scalar=1.0 / 6, in1=e0[:],
                            op0=MUL, op1=ADD,
                        )
                        nc.vector.scalar_tensor_tensor(
                            out=ot3[:, 1, :], in0=ps[:], scalar=1.0 / 3, in1=e1[:],
                            op0=MUL, op1=ADD,
                        )
                        nc.vector.scalar_tensor_tensor(
                            out=e2[:], in0=ps[:], scalar=2.0 / 3, in1=e2[:],
                            op0=MUL, op1=ADD,
                        )
                    else:
                        nc.vector.scalar_tensor_tensor(
                            out=ot3[:, 2, :], in0=ps[:], scalar=1.0, in1=e2[:],
                            op0=MUL, op1=ADD,
                        )
                # DMA out: valid i rows only
                if g < NBLK - 1:
                    nrow = P
                    o_slice = o_d[b, M3 * i0 : M3 * (i0 + nrow), :]
                    nc.sync.dma_start(
                        out=o_slice.rearrange("(i u) c -> i u c", u=M3),
                        in_=ot3[:nrow, :, :],
                    )
                else:
                    nrow = NI_OUT - 1 - i0  # 85 full rows (i <= 1364)
                    o_slice = o_d[b, M3 * i0 : M3 * (i0 + nrow), :]
                    nc.sync.dma_start(
                        out=o_slice.rearrange("(i u) c -> i u c", u=M3),
                        in_=ot3[:nrow, :, :],
                    )
                    # tail: t = 4095 = 3*1365 + 0
                    nc.sync.dma_start(
                        out=o_d[b, L - 1 : L, :],
                        in_=ot3[nrow : nrow + 1, 0, :],
                    )

            def emit_transform_batch(b, start_wp, n_wp):
                for wp in range(start_wp, min(start_wp + n_wp, NWIN // 2)):
                    emit_transform_pair(b, wp)

            def body(first, last):
                for b in range(B_LOC):
                    if first and b == 0:
                        emit_transform_batch(0, 0, NWIN // 2)
                    nxt = (b + 1) % B_LOC
                    skip_next = last and b == B_LOC - 1
                    for g in range(NBLK):
                        emit_block(b, g)
                        if not skip_next:
                            # 18 transform pairs spread over 11 blocks
                            w0 = (18 * g) // NBLK
                            w1 = (18 * (g + 1)) // NBLK
                            emit_transform_batch(nxt, w0, w1 - w0)

            for r in range(repeat):
                body(first=(r == 0), last=(r == repeat - 1))

    nc.compile()
    return nc


def _get_program(repeat=1):
    key = ("nc", repeat)
    if key not in _cache:
        _cache[key] = _build_program(repeat)
    return _cache[key]


def _host_prep(x, w, bias):
    import ml_dtypes

    bf = ml_dtypes.bfloat16
    # x^T: pad to [B, TX, C_IN] bf16, t-major
    xpad = np.zeros((B, TX, C_IN), np.float32)
    xpad[:, PAD : PAD + L, :] = np.transpose(x, (0, 2, 1))
    xt = xpad.astype(bf)
    # U[p,j,c,ci,co] = sum_k G[p,k] w[co, ci, 3j+k]
    wr = w.reshape(C_OUT, C_IN, NJ, 3).astype(np.float64)
    U = np.einsum("pk,ocjk->pjco", G_W, wr)  # (5, 3, C_IN, C_OUT)
    U = U.reshape(NP5, NJ, CI_CHUNKS, P, C_OUT).astype(bf)
    # S[t_local, p*42+i_loc] = BT[p, t_local - 3*i_loc]
    S = np.zeros((P, SCOLS), np.float64)
    for pp in range(NP5):
        for il in range(WTILES):
            for q in range(NP5):
                t_local = 3 * il + q
                if t_local < P:
                    S[t_local, pp * WTILES + il] = BT_W[pp, q]
    S = S.astype(bf)
    return xt, U, S, bias.astype(np.float32)


def _make_in_maps(x, w, bias):
    xt, U, S, bias_f = _host_prep(
        np.asarray(x, np.float32), np.asarray(w, np.float32),
        np.asarray(bias, np.float32),
    )
    return [
        {
            "xt": np.ascontiguousarray(xt[c * B_LOC : (c + 1) * B_LOC]),
            "u": U,
            "s": S,
            "bias": bias_f,
        }
        for c in range(N_CORES)
    ]


def _get_runner():
    """Cached SPMD runner: same bass2jax/PJRT execution path that
    run_bass_kernel_spmd uses under axon, but the jitted executable and the
    (constant) zero output operands are built once and reused per call."""
    if "runner" in _cache:
        return _cache["runner"]

    import jax
    from jax.sharding import Mesh, NamedSharding, PartitionSpec
    from jax.experimental.shard_map import shard_map
    from concourse import mybir
    from concourse.bass2jax import (
        _bass_exec_p,
        install_neuronx_cc_hook,
        partition_id_tensor,
    )

    install_neuronx_cc_hook()
    nc = _get_program()
    partition_name = nc.partition_id_tensor.name if nc.partition_id_tensor else None
    in_names, out_names, out_avals, zero_outs = [], [], [], []
    for alloc in nc.m.functions[0].allocations:
        if not isinstance(alloc, mybir.MemoryLocationSet):
            continue
        name = alloc.memorylocations[0].name
        if alloc.kind == "ExternalInput":
            if name != partition_name:
                in_names.append(name)
        elif alloc.kind == "ExternalOutput":
            shape = tuple(alloc.tensor_shape)
            dtype = mybir.dt.np(alloc.dtype)
            out_names.append(name)
            out_avals.append(jax.core.ShapedArray(shape, dtype))
            zero_outs.append(np.zeros(shape, dtype))
    n_params = len(in_names)
    all_names = in_names + out_names
    if partition_name is not None:
        all_names = all_names + [partition_name]

    def _body(*args):
        extra = [partition_id_tensor()] if partition_name is not None else []
        return tuple(
            _bass_exec_p.bind(
                *(list(args) + extra),
                out_avals=tuple(out_avals),
                in_names=tuple(all_names),
                out_names=tuple(out_names),
                lowering_input_output_aliases=(),
                sim_require_finite=True,
                sim_require_nnan=True,
                nc=nc,
            )
        )

    devices = jax.devices()[:N_CORES]
    mesh = Mesh(np.asarray(devices), ("core",))
    sharding = NamedSharding(mesh, PartitionSpec("core"))
    fn = jax.jit(
        shard_map(
            _body,
            mesh=mesh,
            in_specs=(PartitionSpec("core"),) * (n_params + len(out_names)),
            out_specs=(PartitionSpec("core"),) * len(out_names),
            check_rep=False,
        )
    )
    zeros_dev = [
        jax.device_put(np.concatenate([z] * N_CORES, axis=0), sharding)
        for z in zero_outs
    ]
    _cache["runner"] = (fn, in_names, out_names, zeros_dev, sharding)
    return _cache["runner"]


def kernel(**inputs):
    x = np.asarray(inputs["x"], dtype=np.float32)
    w = np.asarray(inputs["weight"], dtype=np.float32)
    bias = np.asarray(inputs["bias"], dtype=np.float32)

    try:
        import jax

        fn, in_names, out_names, zeros_dev, sharding = _get_runner()
        in_maps = _make_in_maps(x, w, bias)
        glob = {
            nm: np.concatenate([np.asarray(in_maps[c][nm]) for c in range(N_CORES)], axis=0)
            for nm in in_names
        }
        dev_in = [jax.device_put(glob[nm], sharding) for nm in in_names]
        r = fn(*dev_in, *zeros_dev)
        out = np.asarray(r[out_names.index("out")])
        return out.reshape(B, L, C_OUT)
    except Exception:
        # Fallback: the stock SPMD runner (same program, per-core in_maps).
        from concourse.bass_utils import run_bass_kernel_spmd

        nc = _get_program()
        res = run_bass_kernel_spmd(
            nc, _make_in_maps(x, w, bias), list(range(N_CORES))
        )
        return np.concatenate(
            [res.results[c]["out"] for c in range(N_CORES)], axis=0
        )
